# revision 25
# baseline (speedup 1.0000x reference)
"""Bass/Trainium2 kernel for nn_BehaviorSpecificPFF (MoE-style routed FFN).

Reference semantics (per token t):
    e = b_seq[t]
    out[t] = 0                                   if e == 0
    out[t] = relu(x[t] @ W1[e-1] + b1[e-1]) @ W2[e-1] + b2[e-1]   otherwise

Strategy (data parallel over batch: 4 batches per core on 8 cores; per core,
8192 tokens, everything on device):
  1. Routing scan over b_seq (column-wrapped token order t = j*128 + p):
     per-token slot in per-expert buckets via in-row Hillis-Steele prefix +
     matmul cross-partition prefix. Expert-0 tokens map to a shared trash
     slot. perm (token -> slot) is then shuffled on-chip into the
     16-partition-wrapped int16 index layout the SWDGE custom DMA ops expect
     (one [128,64] PE transpose + 8 [64,16] PE transposes + strided DVE
     copies + 8 partition-group replication DMAs).
  2. Dispatch: one dma_scatter_add scatters all 8192 x rows (bf16) into
     expert-sorted xsorted DRAM using perm16. Padding slots stay garbage
     (their columns are computed and discarded); expert-0 rows pile into a
     trash slot.
  3. Per expert supertile of up to 512 slots: dma_gather(transpose=True) with
     constant consecutive indices loads xsorted directly into [d-chunk, token]
     layout; two bf16 matmul layers (fp32 PSUM) with fused bias+relu;
     PE-transpose back to [token, d]; contiguous store into ysorted.
  4. Return: one dma_gather (non-transpose) reads ysorted[perm[t]] for every
     token (expert-0 tokens hit the zeroed trash region) and a contiguous
     store writes y. Host casts bf16 -> fp32.
  Bucket capacities are specialized per call (max over cores, rounded to 128).
"""

import numpy as np
import ml_dtypes

import concourse.bass as bass
import concourse.tile as tile
from concourse import bacc, mybir
from concourse.bass_utils import run_bass_kernel_spmd
from concourse.masks import make_identity

N_CORES = 8
B, T, D, DFF, NB = 32, 2048, 256, 1024, 4
P = 128
NTOK = B * T // N_CORES          # 8192 tokens per core
JCOL = NTOK // P                 # 64 scan columns
F32 = mybir.dt.float32
BF16 = mybir.dt.bfloat16
I32 = mybir.dt.int32
I16 = mybir.dt.int16
AF = mybir.ActivationFunctionType
ALU = mybir.AluOpType


def build_nc(caps, debug=False, reps=1, parts=("scan", "gather", "mm", "scatter")):
    """caps: [cap_e1..cap_e4], each a multiple of 128."""
    rcaps = list(caps[:NB])
    ntiles = [c // P for c in rcaps]
    nslot = sum(rcaps)
    bases = [sum(rcaps[:e]) for e in range(NB)]
    nslot_e = nslot + P              # + trash/zero region for expert 0

    nc = bacc.Bacc("TRN2", target_bir_lowering=False, debug=False,
                   num_devices=N_CORES)
    xs_d = nc.dram_tensor("xsb", [P, JCOL * D], BF16, kind="ExternalInput").ap()
    b_d = nc.dram_tensor("b", [NTOK], I32, kind="ExternalInput").ap()
    w1_d = nc.dram_tensor("w1s", [P, 2 * NB * DFF], BF16, kind="ExternalInput").ap()
    w2_d = nc.dram_tensor("w2s", [P, (DFF // P) * NB * D], BF16, kind="ExternalInput").ap()
    b1_d = nc.dram_tensor("b1s", [P, NB * (DFF // P)], F32, kind="ExternalInput").ap()
    b2_d = nc.dram_tensor("b2s", [P, NB * (D // P)], F32, kind="ExternalInput").ap()
    cgi_d = nc.dram_tensor("cgi", [P, nslot // 16], I16, kind="ExternalInput").ap()
    y_d = nc.dram_tensor("y", [NTOK, D], BF16, kind="ExternalOutput").ap()
    # ExternalOutput => PJRT pre-zeros (donated zero buffer); scatter_add is
    # "+=", so the real slots need a zero base.
    xsort = nc.dram_tensor("xsort", [nslot_e, D], BF16, kind="ExternalOutput").ap()
    ysort = nc.dram_tensor("ysort", [nslot_e, D], BF16, kind="Internal").ap()
    dbg = (nc.dram_tensor("dbg", [P, 512], I16, kind="ExternalOutput").ap()
           if debug else None)

    with tile.TileContext(nc) as tc:
        _body(tc, xs_d, b_d, w1_d, w2_d, b1_d, b2_d, cgi_d, y_d, xsort, ysort,
              rcaps, ntiles, bases, nslot, reps, parts, dbg)
    nc.compile()
    return nc


def _body(tc, xs_d, b_d, w1_d, w2_d, b1_d, b2_d, cgi_d, y_d, xsort, ysort,
          rcaps, ntiles, bases, nslot, reps=1,
          parts=("scan", "gather", "mm", "scatter"), dbg=None):
    nc = tc.nc
    nv = nc.vector
    ng = nc.gpsimd
    sy = nc.sync

    import contextlib
    ctx = contextlib.ExitStack()
    with ctx:
        const = ctx.enter_context(tc.tile_pool(name="const", bufs=1))
        scan = ctx.enter_context(tc.tile_pool(name="scan", bufs=1))
        xtp = ctx.enter_context(tc.tile_pool(name="xt", bufs=5))
        htp = ctx.enter_context(tc.tile_pool(name="ht", bufs=12))
        ytp = ctx.enter_context(tc.tile_pool(name="yt", bufs=6))
        yop = ctx.enter_context(tc.tile_pool(name="yo", bufs=4))
        ysp = ctx.enter_context(tc.tile_pool(name="ys", bufs=1))
        ps_h = ctx.enter_context(tc.tile_pool(name="ps_h", bufs=2, space="PSUM"))
        ps_y = ctx.enter_context(tc.tile_pool(name="ps_y", bufs=2, space="PSUM"))
        ps_t = ctx.enter_context(tc.tile_pool(name="ps_t", bufs=1, space="PSUM"))
        ps_s = ctx.enter_context(tc.tile_pool(name="ps_s", bufs=1, space="PSUM"))

        # ---- constants / weights -------------------------------------------
        identb = const.tile([P, P], BF16)
        make_identity(nc, identb[:])
        identf = const.tile([P, P], F32)
        make_identity(nc, identf[:])
        ltri = const.tile([P, P], F32)                 # ltri[k, m] = 1 if k < m
        ng.memset(ltri[:], 1.0)
        ng.affine_select(out=ltri[:], in_=ltri[:], compare_op=ALU.is_gt,
                         fill=0.0, base=0, pattern=[[1, P]], channel_multiplier=-1)

        w1s = const.tile([P, 2 * NB * DFF], BF16)
        sy.dma_start(w1s[:], w1_d[:])
        w2s = const.tile([P, (DFF // P) * NB * D], BF16)
        sy.dma_start(w2s[:], w2_d[:])
        b1s = const.tile([P, NB * (DFF // P)], F32)
        sy.dma_start(b1s[:], b1_d[:])
        b2s = const.tile([P, NB * (D // P)], F32)
        sy.dma_start(b2s[:], b2_d[:])
        cgi = const.tile([P, nslot // 16], I16)
        sy.dma_start(cgi[:], cgi_d[:])
        # x resident in SBUF, token t = j*128+p at [p, j*D:(j+1)*D]
        x_sb = const.tile([P, JCOL * D], BF16)
        sy.dma_start(x_sb[:], xs_d[:])
        # zero the trash region of ysort (expert-0 tokens gather from there)
        zt = const.tile([P, D], BF16)
        ng.memset(zt[:], 0.0)
        sy.dma_start(ysort[nslot:nslot + P, :].rearrange("(o p) d -> p o d", p=P),
                     zt[:].rearrange("p (o d) -> p o d", o=1))

        idxT = const.tile([P, JCOL * 8], I16)   # perm16, idx i at [i%16, i//16]
        if "scan" not in parts:
            ng.memset(idxT[:], 0)

        for _rep in range(reps):
            if "scan" in parts:
                _scan_phase(tc, b_d, bases, nslot, scan, ps_s, ltri, identf,
                            idxT, dbg)
            _ffn_phase(tc, x_sb, y_d, xsort, ysort, cgi, idxT,
                       rcaps, ntiles, bases, nslot,
                       xtp, htp, ytp, yop, ysp, ps_h, ps_y, ps_t,
                       identb, w1s, w2s, b1s, b2s, parts)


def _scan_phase(tc, b_d, bases, nslot, scan, ps_s, ltri, identf, idxT, dbg=None):
    nc = tc.nc
    nv = nc.vector
    sy = nc.sync

    # column-wrapped token order: b_i[p, j] = b_seq[j*128 + p]
    b_i = scan.tile([P, JCOL], I32)
    sy.dma_start(b_i[:], b_d.rearrange("(j p) -> p j", p=P))
    b_f = scan.tile([P, JCOL], F32)
    nv.tensor_copy(b_f[:], b_i[:])

    M = scan.tile([P, NB * JCOL], F32)
    M3 = M[:].rearrange("p (e j) -> p e j", e=NB)
    for e in range(NB):
        nv.tensor_scalar(M3[:, e, :], b_f[:], float(e + 1), None, ALU.is_equal)

    # in-row inclusive prefix sum along j (Hillis-Steele, ping-pong)
    sA = scan.tile([P, NB * JCOL], F32)
    sB = scan.tile([P, NB * JCOL], F32)
    cur, nxt = M, sA
    s = 1
    while s < JCOL:
        c3 = cur[:].rearrange("p (e j) -> p e j", e=NB)
        n3 = nxt[:].rearrange("p (e j) -> p e j", e=NB)
        nv.tensor_copy(n3[:, :, 0:s], c3[:, :, 0:s])
        nv.tensor_add(n3[:, :, s:JCOL], c3[:, :, s:JCOL], c3[:, :, 0:JCOL - s])
        cur = nxt
        nxt = sB if cur is sA else sA
        s *= 2
    incl = cur

    # per-row totals and cross-partition exclusive prefix (via matmul)
    cnt = scan.tile([P, NB], F32)
    nv.tensor_reduce(cnt[:],
                     incl[:].rearrange("p (e j) -> p e j", e=NB)[:, :, JCOL - 1:JCOL],
                     mybir.AxisListType.X, ALU.add)
    exr_ps = ps_s.tile([P, NB], F32, tag="exr", name="exr_ps")
    nc.tensor.matmul(exr_ps[:], ltri[:], cnt[:], start=True, stop=True)
    exr = scan.tile([P, NB], F32)
    nv.tensor_copy(exr[:], exr_ps[:])

    cand = scan.tile([P, NB * JCOL], F32)
    c3 = cand[:].rearrange("p (e j) -> p e j", e=NB)
    i3 = incl[:].rearrange("p (e j) -> p e j", e=NB)
    for e in range(NB):
        nv.tensor_scalar(c3[:, e, :], i3[:, e, :], exr[:, e:e + 1],
                         float(bases[e] - 1), ALU.add, ALU.add)
    prod = scan.tile([P, NB * JCOL], F32)
    nv.tensor_tensor(out=prod[:], in0=M[:], in1=cand[:], op=ALU.mult)
    perm_f = scan.tile([P, JCOL], F32)
    nv.tensor_reduce(perm_f[:],
                     prod[:].rearrange("p (e j) -> p j e", e=NB),
                     mybir.AxisListType.X, ALU.add)
    # expert-0 tokens -> shared trash slot nslot
    m0s = scan.tile([P, JCOL], F32)
    nv.tensor_scalar(m0s[:], b_f[:], 0.0, float(nslot), ALU.is_equal, ALU.mult)
    nv.tensor_add(perm_f[:], perm_f[:], m0s[:])

    # ---- shuffle perm into the 16-wrapped idx layout -----------------------
    # token i = j*128 + p; target idxT[i%16, i//16] = idxT[p%16, j*8 + p//16]
    ptr_ps = ps_s.tile([64, P], F32, tag="ptr", name="ptr_ps")
    nc.tensor.transpose(out=ptr_ps[:], in_=perm_f[:], identity=identf[:])
    permT = scan.tile([64, P], F32)
    nv.tensor_copy(permT[:], ptr_ps[:])
    idx3 = idxT[:].rearrange("q (j u) -> q u j", u=8)
    for u in range(8):
        psu = ps_s.tile([16, JCOL], F32, tag="psu", name="psu")
        nc.tensor.transpose(out=psu[:], in_=permT[:, 16 * u:16 * u + 16],
                            identity=identf[0:64, 0:64])
        nv.tensor_copy(idx3[:16, u, :], psu[:])
    if dbg is not None:
        sy.dma_start(dbg[:, :], idxT[:])
    # replicate partition group 0 into groups 1..7
    for u in range(1, 8):
        sy.dma_start(idxT[16 * u:16 * u + 16, :], idxT[0:16, :])


def _ffn_phase(tc, x_sb, y_d, xsort, ysort, cgi, idxT,
               rcaps, ntiles, bases, nslot,
               xtp, htp, ytp, yop, ysp, ps_h, ps_y, ps_t,
               identb, w1s, w2s, b1s, b2s, parts):
    nc = tc.nc
    nv = nc.vector
    ns = nc.scalar
    ng = nc.gpsimd
    sy = nc.sync

    # dispatch: scatter all x rows to their expert-sorted slots
    # (16 x 512-index chunks -- the HW-validated scatter_add shape)
    if "gather" in parts:
        x3 = x_sb[:].rearrange("p (j d) -> p j d", d=D)
        for c in range(JCOL // 4):
            ng.dma_scatter_add(
                xsort[:],
                x3[:, 4 * c:4 * c + 4, :],
                idxT[:, 32 * c:32 * c + 32],
                512, 512, D)

    tiles = []
    for e in range(NB):
        g0 = 0
        while g0 < ntiles[e]:
            G = min(4, ntiles[e] - g0)
            tiles.append((e, bases[e] // P + g0, G))
            g0 += G

    FETCH_AHEAD = 3
    store = {}

    def fetch(i):
        if i >= len(tiles):
            return
        _, t0, G = tiles[i]
        ntoks = G * P
        xt = xtp.tile([P, 2 * 512], BF16, name="xt")
        if "gather" in parts:
            ng.dma_gather(
                xt[:, :2 * ntoks].rearrange("p (k t) -> p k t", k=2),
                xsort[:],
                cgi[:, t0 * 8:t0 * 8 + 8 * G],
                ntoks, ntoks, D,
                transpose=True)
        elif "mm" in parts:
            nv.memset(xt[:], 0.0)
        store[i] = xt

    for i in range(min(FETCH_AHEAD, len(tiles))):
        fetch(i)

    for i, (e, t0, G) in enumerate(tiles):
        ntoks = G * P
        xt = store.pop(i)

        yo = yop.tile([P, 4 * D], BF16)
        if "scatter" in parts and "mm" not in parts:
            nv.memset(yo[:], 0.0)
        if "mm" in parts:
            # layer 1 + fused bias/relu -> ht[m][dff_chunk, tok]
            ht = [htp.tile([P, 512], BF16, tag="ht", name="ht")
                  for _ in range(DFF // P)]
            for m in range(DFF // P):
                hps = ps_h.tile([P, 512], F32)
                for k in range(2):
                    nc.tensor.matmul(
                        hps[:, :ntoks],
                        w1s[:, (e * 2 + k) * DFF + m * P:(e * 2 + k) * DFF + (m + 1) * P],
                        xt[:, k * ntoks:(k + 1) * ntoks],
                        start=(k == 0), stop=(k == 1))
                if m % 2 == 0:
                    ns.activation(ht[m][:, :ntoks], hps[:, :ntoks], AF.Relu,
                                  bias=b1s[:, e * (DFF // P) + m:e * (DFF // P) + m + 1],
                                  scale=1.0)
                else:
                    nv.tensor_scalar(ht[m][:, :ntoks], hps[:, :ntoks],
                                     b1s[:, e * (DFF // P) + m:e * (DFF // P) + m + 1],
                                     0.0, ALU.add, ALU.max)

            # layer 2 + bias -> yt[c][dmodel_chunk, tok]
            yt = [ytp.tile([P, 512], BF16, tag="yt", name="yt")
                  for _ in range(D // P)]
            for c in range(D // P):
                yps = ps_y.tile([P, 512], F32)
                for k in range(DFF // P):
                    nc.tensor.matmul(
                        yps[:, :ntoks],
                        w2s[:, (e * (DFF // P) + k) * D + c * P:(e * (DFF // P) + k) * D + (c + 1) * P],
                        ht[k][:, :ntoks],
                        start=(k == 0), stop=(k == DFF // P - 1))
                nv.tensor_scalar(yt[c][:, :ntoks], yps[:, :ntoks],
                                 b2s[:, e * (D // P) + c:e * (D // P) + c + 1],
                                 None, ALU.add)

            # transpose back [dmodel, tok] -> yo[tok, dmodel], 2 blocks/pack
            for pk in range((G + 1) // 2):
                gis = [gi for gi in (2 * pk, 2 * pk + 1) if gi < G]
                pso = ps_t.tile([P, 512], BF16, tag="pst", name="pso")
                for bi, gi in enumerate(gis):
                    for c in range(D // P):
                        nc.tensor.transpose(
                            out=pso[:, bi * D + c * P: bi * D + (c + 1) * P],
                            in_=yt[c][:, gi * P:(gi + 1) * P],
                            identity=identb[:])
                nv.tensor_copy(yo[:, 2 * pk * D: (2 * pk + len(gis)) * D],
                               pso[:, :len(gis) * D])

        # prefetch two supertiles ahead before this tile's store
        fetch(i + FETCH_AHEAD)

        if "scatter" in parts:
            # contiguous store: slot t0*128 + g*128 + p <- yo[p, g]
            sy.dma_start(
                ysort[t0 * P:t0 * P + ntoks, :].rearrange("(g p) d -> p g d", p=P),
                yo[:, :G * D].rearrange("p (g d) -> p g d", g=G))

    if "scatter" in parts:
        # un-permute: y[t] = ysort[perm[t]] for every token
        ysb = ysp.tile([P, JCOL * D], BF16, tag="ysb", name="ysb")
        y3 = ysb[:].rearrange("p (j d) -> p j d", d=D)
        for c in range(JCOL // 4):
            ng.dma_gather(
                y3[:, 4 * c:4 * c + 4, :],
                ysort[:],
                idxT[:, 32 * c:32 * c + 32],
                512, 512, D,
                transpose=False)
        sy.dma_start(y_d.rearrange("(j p) d -> p j d", p=P),
                     ysb[:].rearrange("p (j d) -> p j d", d=D))


def prep_inputs(x, W1, b1, W2, b2, b_seq):
    """Shard + pre-layout host-side. Returns (in_maps, caps)."""
    x = np.asarray(x, dtype=np.float32)
    W1 = np.asarray(W1, dtype=np.float32)
    b1 = np.asarray(b1, dtype=np.float32)
    W2 = np.asarray(W2, dtype=np.float32)
    b2 = np.asarray(b2, dtype=np.float32)
    b_seq = np.ascontiguousarray(np.asarray(b_seq, dtype=np.int32))

    w1s = np.ascontiguousarray(
        W1.reshape(NB, 2, P, DFF).transpose(2, 0, 1, 3).reshape(P, 2 * NB * DFF)
    ).astype(ml_dtypes.bfloat16)
    w2s = np.ascontiguousarray(
        W2.reshape(NB, DFF // P, P, D).transpose(2, 0, 1, 3).reshape(P, -1)
    ).astype(ml_dtypes.bfloat16)
    b1s = np.ascontiguousarray(
        b1.reshape(NB, DFF // P, P).transpose(2, 0, 1).reshape(P, -1))
    b2s = np.ascontiguousarray(
        b2.reshape(NB, D // P, P).transpose(2, 0, 1).reshape(P, -1))

    bpc = B // N_CORES
    in_maps = []
    counts = np.zeros((N_CORES, NB), dtype=np.int64)
    for c in range(N_CORES):
        xc = x[c * bpc:(c + 1) * bpc].reshape(NTOK, D).astype(ml_dtypes.bfloat16)
        # token t = j*128 + p at [p, j*D:(j+1)*D]
        xsb = np.ascontiguousarray(
            xc.reshape(JCOL, P, D).transpose(1, 0, 2).reshape(P, JCOL * D))
        bc = b_seq[c * bpc:(c + 1) * bpc].reshape(NTOK)
        for e in range(NB):
            counts[c, e] = int((bc == e + 1).sum())
        in_maps.append({"xsb": xsb, "b": np.ascontiguousarray(bc),
                        "w1s": w1s, "w2s": w2s, "b1s": b1s, "b2s": b2s})
    caps = [max(P, int(np.ceil(counts[:, e].max() / P)) * P) for e in range(NB)]
    nslot = sum(caps)
    # constant consecutive gather indices: idx i = slot i at [i%16, i//16]
    cg = np.zeros((16, nslot // 16), np.int16)
    for i in range(nslot):
        cg[i % 16, i // 16] = i
    cgi = np.ascontiguousarray(np.tile(cg, (8, 1)))
    for m in in_maps:
        m["cgi"] = cgi
    return in_maps, caps


def assemble(results):
    bpc = B // N_CORES
    out = np.empty((B, T, D), dtype=np.float32)
    for c in range(N_CORES):
        out[c * bpc:(c + 1) * bpc] = (
            results[c]["y"].astype(np.float32).reshape(bpc, T, D))
    return out


def kernel(x, W1, b1, W2, b2, b_seq):
    in_maps, caps = prep_inputs(x, W1, b1, W2, b2, b_seq)
    nc = build_nc(caps)
    res = run_bass_kernel_spmd(nc, in_maps, core_ids=list(range(N_CORES)))
    return assemble(res.results)
